# revision 1
# baseline (speedup 1.0000x reference)
"""ATACSeq RBF-embedding kernel for 8 Trainium2 NeuronCores.

Math (per sample b with chromosome k = chrom[b]):
    w[n]  = exp(-(pos_b - centers[k,n])^2 / (2 * exp(logvar[k,n])))
    out_b = (w / w.sum()) @ embeddings[k]          # [D]

Sharding: samples are grouped by chromosome on the host; core i owns
chromosomes [3i, 3i+3) and receives ONLY those embedding matrices
(3 x 1 MB instead of the full 25 MB stack) plus its grouped, padded
positions.  All per-sample math runs on-device:

  - one tiny PE matmul per sample-chunk broadcasts positions across
    the 128 partitions (lhsT = ones row),
  - a fused DVE tensor_scalar computes diff = p*t - c*t with the
    variance fold t = sqrt(1/(2v)) precomputed per center (two
    per-partition scalar operands, one instruction),
  - DVE square + ACT exp(-x) -> unnormalized w [n, b] written as
    float32r (the rounding producer the fp32r matmuls require),
  - weighted sum: w.T @ E accumulated over 4 partition chunks of n in
    PSUM, float32r (single-pass full-rate fp32 matmul),
  - normalizer: w.T @ ones-matrix -> column-form Z without any
    transpose; reciprocal + per-partition scale of the PSUM result.

The host then scatters each core's [3, CAP, D] block back into the
full [B, D] output using the sort permutation.
"""

import math
import sys
import types

import numpy as np

import concourse.bass as bass
import concourse.tile as tile
from concourse import bacc, mybir
from concourse.bass_utils import run_bass_kernel_spmd


def _ensure_ntff_hook():
    """Provide antenv.axon_hooks if the container's antenv stub lacks it.

    bass_utils' BASS_TRACE=1 path imports antenv.axon_hooks to fetch the
    NTFF profile hook; the trimmed antenv in this container doesn't ship
    it.  Register a minimal holder backed by trn_agent_boot's ctypes
    shim.  Fully guarded: on any failure tracing is simply unavailable.
    """
    try:
        import antenv.axon_hooks  # noqa: F401

        return
    except Exception:
        pass
    try:
        import antenv

        mod = types.ModuleType("antenv.axon_hooks")
        holder = [None, False]

        def set_axon_ntff_profile_hook(h):
            holder[0] = h
            holder[1] = True

        def get_axon_ntff_profile_hook():
            if not holder[1]:
                holder[1] = True
                try:
                    from trn_agent_boot.trn_boot import (
                        _ntff_profile_via_ctypes,
                    )

                    holder[0] = _ntff_profile_via_ctypes(
                        "/opt/axon/libaxon_pjrt.so"
                    )
                except Exception:
                    holder[0] = None
            return holder[0]

        mod.set_axon_ntff_profile_hook = set_axon_ntff_profile_hook
        mod.get_axon_ntff_profile_hook = get_axon_ntff_profile_hook
        sys.modules["antenv.axon_hooks"] = mod
        antenv.axon_hooks = mod
    except Exception:
        pass


_ensure_ntff_hook()

N_CORES = 8
P = 128  # SBUF partitions

# Filled in by kernel() on every call so a harness/test can inspect the
# BassKernelResults of the last run (exec_time_ns etc. when BASS_TRACE=1).
LAST_RESULTS = None

_NC_CACHE = {}


def _build_nc(KPC: int, N: int, D: int, CAP: int, SC: int):
    """Build the SPMD Bass module (identical program for every core).

    Per-core DRAM I/O:
      emb [KPC, N, D] f32r  core's embedding matrices
      tcp [P, KPC, 2, NCH]  partition-major t / c*t per center chunk
      pos [1, KPC*CAP] f32  grouped positions, padded with 0
      out [KPC, CAP, D] f32 per-sample outputs (padded rows are garbage)
    """
    f32 = mybir.dt.float32
    f32r = mybir.dt.float32r
    NCH = N // P          # chunks of the center dim (4 for N=512)
    n_sc = CAP // SC      # sample chunks of <=128 samples

    nc = bacc.Bacc("TRN2", target_bir_lowering=False, debug=False)
    emb = nc.dram_tensor("emb", [KPC, N, D], f32r, kind="ExternalInput").ap()
    tcp = nc.dram_tensor(
        "tcp", [P, KPC, 2, NCH], f32, kind="ExternalInput"
    ).ap()
    pos = nc.dram_tensor("pos", [1, KPC * CAP], f32, kind="ExternalInput").ap()
    out = nc.dram_tensor("out", [KPC, CAP, D], f32, kind="ExternalOutput").ap()

    mult = mybir.AluOpType.mult
    subtract = mybir.AluOpType.subtract

    with tile.TileContext(nc) as tc:
        with (
            tc.tile_pool(name="consts", bufs=1) as consts,
            tc.tile_pool(name="embp", bufs=3) as embp,
            tc.tile_pool(name="wp", bufs=3) as wp,
            tc.tile_pool(name="small", bufs=3) as small,
            tc.tile_pool(name="ps_bc", bufs=2, space="PSUM") as ps_bc,
            tc.tile_pool(name="ps_out", bufs=2, space="PSUM") as ps_out,
        ):
            # Constants + small inputs first (SWDGE via gpsimd so they
            # never queue behind the 1 MB embedding DMAs on sync HWDGE).
            ones_row = consts.tile([1, P], f32)
            nc.vector.memset(ones_row, 1.0)
            ones_f32 = consts.tile([P, SC], f32)
            nc.vector.memset(ones_f32, 1.0)
            ones_mat = consts.tile([P, SC], f32r)
            nc.vector.tensor_copy(ones_mat, ones_f32)

            tcp_sb = consts.tile([P, KPC, 2, NCH], f32)
            nc.gpsimd.dma_start(out=tcp_sb, in_=tcp)
            pos_sb = consts.tile([1, KPC * CAP], f32)
            nc.gpsimd.dma_start(out=pos_sb, in_=pos)

            # Embedding DMAs up-front (bufs=3 keeps all three resident).
            e_sbs = []
            for k in range(KPC):
                e_sb = embp.tile([P, NCH, D], f32r, tag="e")
                nc.sync.dma_start(
                    out=e_sb, in_=emb[k].rearrange("(c p) d -> p c d", p=P)
                )
                e_sbs.append(e_sb)

            for k in range(KPC):
                e_sb = e_sbs[k]
                for c in range(n_sc):
                    # Broadcast this chunk's positions to all partitions.
                    p_bc = ps_bc.tile([P, SC], f32, tag="pbc")
                    nc.tensor.matmul(
                        out=p_bc,
                        lhsT=ones_row,
                        rhs=pos_sb[:, k * CAP + c * SC : k * CAP + (c + 1) * SC],
                        start=True,
                        stop=True,
                    )
                    # diff = p*t - c*t, squared, exp(-x) -> w (float32r)
                    diff = wp.tile([P, NCH * SC], f32, tag="diff")
                    for j in range(NCH):
                        nc.vector.tensor_scalar(
                            out=diff[:, j * SC : (j + 1) * SC],
                            in0=p_bc,
                            scalar1=tcp_sb[:, k, 0, j : j + 1],
                            scalar2=tcp_sb[:, k, 1, j : j + 1],
                            op0=mult,
                            op1=subtract,
                        )
                    sq = wp.tile([P, NCH * SC], f32, tag="sq")
                    nc.vector.tensor_mul(sq, diff, diff)
                    w = wp.tile([P, NCH * SC], f32r, tag="w")
                    nc.scalar.activation(
                        out=w,
                        in_=sq,
                        func=mybir.ActivationFunctionType.Exp,
                        scale=-1.0,
                    )
                    o_ps = ps_out.tile([SC, D], f32, tag="o")
                    z_ps = ps_out.tile([SC, SC], f32, tag="z")
                    for j in range(NCH):
                        nc.tensor.matmul(
                            out=o_ps,
                            lhsT=w[:, j * SC : (j + 1) * SC],
                            rhs=e_sb[:, j, :],
                            start=(j == 0),
                            stop=(j == NCH - 1),
                        )
                    # Column-form normalizer: every column of z_ps is Z.
                    for j in range(NCH):
                        nc.tensor.matmul(
                            out=z_ps,
                            lhsT=w[:, j * SC : (j + 1) * SC],
                            rhs=ones_mat,
                            start=(j == 0),
                            stop=(j == NCH - 1),
                        )
                    rz = small.tile([SC, 1], f32, tag="rz")
                    nc.vector.reciprocal(rz, z_ps[:, 0:1])
                    o_sb = wp.tile([SC, D], f32, tag="osb")
                    nc.vector.tensor_scalar_mul(o_sb, o_ps, rz)
                    nc.sync.dma_start(
                        out=out[k, c * SC : (c + 1) * SC, :], in_=o_sb
                    )
    nc.compile()
    return nc


def _get_nc(KPC, N, D, CAP, SC):
    key = (KPC, N, D, CAP, SC)
    if key not in _NC_CACHE:
        _NC_CACHE[key] = _build_nc(*key)
    return _NC_CACHE[key]


def _shard(chromosome, position, embeddings, centers, log_variances):
    """Group samples by chromosome and build per-core input maps."""
    B = chromosome.shape[0]
    K, N, D = embeddings.shape
    KPC = math.ceil(K / N_CORES)
    Kpad = KPC * N_CORES
    NCH = N // P

    counts = np.bincount(chromosome, minlength=Kpad)
    maxc = max(1, int(counts.max()))
    if maxc <= P:
        CAP = max(32, ((maxc + 31) // 32) * 32)
        SC = CAP
    else:
        CAP = ((maxc + P - 1) // P) * P
        SC = P

    order = np.argsort(chromosome, kind="stable")
    starts = np.zeros(Kpad + 1, dtype=np.int64)
    starts[1 : K + 1] = np.cumsum(counts[:K])
    starts[K + 1 :] = starts[K]
    sorted_pos = position[order, 0].astype(np.float32)

    pos_all = np.zeros((Kpad, CAP), dtype=np.float32)
    for k in range(Kpad):
        pos_all[k, : counts[k]] = sorted_pos[starts[k] : starts[k + 1]]

    t = np.sqrt(0.5 * np.exp(-log_variances.astype(np.float64))).astype(
        np.float32
    )
    ct = (centers.astype(np.float32) * t).astype(np.float32)
    # partition-major [P, K, 2, NCH]: [p, k, 0, j] = t[k, j*P+p]
    tcp_all = np.zeros((P, Kpad, 2, NCH), dtype=np.float32)
    tcp_all[:, :K, 0, :] = t.reshape(K, NCH, P).transpose(2, 0, 1)
    tcp_all[:, :K, 1, :] = ct.reshape(K, NCH, P).transpose(2, 0, 1)

    emb_all = np.zeros((Kpad, N, D), dtype=np.float32)
    emb_all[:K] = embeddings

    in_maps = []
    for i in range(N_CORES):
        sl = slice(i * KPC, (i + 1) * KPC)
        in_maps.append(
            {
                "emb": np.ascontiguousarray(emb_all[sl]),
                "tcp": np.ascontiguousarray(tcp_all[:, sl]),
                "pos": np.ascontiguousarray(
                    pos_all[sl].reshape(1, KPC * CAP)
                ),
            }
        )
    meta = (B, D, KPC, CAP, SC, order, starts, counts)
    return in_maps, meta


def kernel(chromosome, position, embeddings, centers, log_variances):
    global LAST_RESULTS
    chromosome = np.asarray(chromosome, dtype=np.int32)
    position = np.asarray(position, dtype=np.float32)
    embeddings = np.asarray(embeddings, dtype=np.float32)
    centers = np.asarray(centers, dtype=np.float32)
    log_variances = np.asarray(log_variances, dtype=np.float32)

    in_maps, meta = _shard(
        chromosome, position, embeddings, centers, log_variances
    )
    B, D, KPC, CAP, SC, order, starts, counts = meta
    N = embeddings.shape[1]

    nc = _get_nc(KPC, N, D, CAP, SC)
    res = run_bass_kernel_spmd(nc, in_maps, core_ids=list(range(N_CORES)))
    LAST_RESULTS = res

    out_full = np.zeros((B, D), dtype=np.float32)
    for i in range(N_CORES):
        o = res.results[i]["out"]  # [KPC, CAP, D]
        for tloc in range(KPC):
            k = i * KPC + tloc
            if k >= len(counts) or counts[k] == 0:
                continue
            idx = order[starts[k] : starts[k + 1]]
            out_full[idx] = o[tloc, : counts[k]]
    return out_full



# revision 4
# speedup vs baseline: 1.5954x; 1.5954x over previous
"""ATACSeq RBF-embedding kernel — raw bacc (manual semaphores), v3.

Same math/host-prep as the tile version (see kernel.py docstring), but
the device program is hand-scheduled with explicit semaphores to avoid
the tile framework's startup/drain overhead:

  SYNC : smalls DMA, 3 embedding DMAs (HWDGE ring 1), final out-wait
  PE   : 3 exponent matmuls -> v0 matmuls -> paired (v1||v2) matmuls
         in separate PE column groups; tiny z matmuls
  ACT  : dummy exp (hoists the ~1.3us table load to t=0), 3 exps,
         normalize-scales (per-partition 1/Z), out DMAs (HWDGE ring 2)
  DVE  : ones memset, reciprocals
  tail : barrier, one sem range-clear, barrier
"""

import math
import sys
import types

import numpy as np
import ml_dtypes

import concourse.bass as bass
from concourse import bacc, mybir
from concourse.bass_utils import run_bass_kernel_spmd
from contextlib import ExitStack


def _ensure_ntff_hook():
    try:
        import antenv.axon_hooks  # noqa: F401

        return
    except Exception:
        pass
    try:
        import antenv

        mod = types.ModuleType("antenv.axon_hooks")
        holder = [None, False]

        def set_axon_ntff_profile_hook(h):
            holder[0] = h
            holder[1] = True

        def get_axon_ntff_profile_hook():
            if not holder[1]:
                holder[1] = True
                try:
                    from trn_agent_boot.trn_boot import (
                        _ntff_profile_via_ctypes,
                    )

                    holder[0] = _ntff_profile_via_ctypes(
                        "/opt/axon/libaxon_pjrt.so"
                    )
                except Exception:
                    holder[0] = None
            return holder[0]

        mod.set_axon_ntff_profile_hook = set_axon_ntff_profile_hook
        mod.get_axon_ntff_profile_hook = get_axon_ntff_profile_hook
        sys.modules["antenv.axon_hooks"] = mod
        antenv.axon_hooks = mod
    except Exception:
        pass


_ensure_ntff_hook()

N_CORES = 8
P = 128
SC = 64
NCH = 4
E_DT = mybir.dt.float8e3      # embedding matrices (DMA halved again)
E_NP = ml_dtypes.float8_e3m4
W_DT = mybir.dt.bfloat16      # RBF weights / ones / outputs

LAST_RESULTS = None
_NC_CACHE = {}


def _build_nc(VPC: int, N: int, D: int):
    f32 = mybir.dt.float32
    f32r = mybir.dt.float32r
    W = NCH * SC
    Exp = mybir.ActivationFunctionType.Exp
    Copy = mybir.ActivationFunctionType.Copy

    nc = bacc.Bacc("TRN2", target_bir_lowering=False, debug=False)
    emb = nc.dram_tensor(
        "emb", [VPC, P, NCH, D], E_DT, kind="ExternalInput"
    ).ap()
    small = nc.dram_tensor(
        "small", [12, VPC, 128 + W], f32r, kind="ExternalInput"
    ).ap()
    out = nc.dram_tensor(
        "out", [VPC, SC, D], W_DT, kind="ExternalOutput"
    ).ap()

    groups = []
    if VPC % 2 == 1:
        groups.append((0,))
        rest = list(range(1, VPC))
    else:
        rest = list(range(VPC))
    for i in range(0, len(rest), 2):
        groups.append(tuple(rest[i : i + 2]))

    s_small = nc.alloc_semaphore("s_small")
    s_emb = [nc.alloc_semaphore(f"s_e{v}") for v in range(VPC)]
    s_misc = nc.alloc_semaphore("s_misc")
    s_arg = nc.alloc_semaphore("s_arg")
    s_peo = nc.alloc_semaphore("s_peo")
    s_act = nc.alloc_semaphore("s_act")
    s_pez = nc.alloc_semaphore("s_pez")
    s_rz = nc.alloc_semaphore("s_rz")
    s_out = nc.alloc_semaphore("s_out")
    sem_lo = s_small.num
    sem_hi = s_out.num

    es = ExitStack()
    with es:
        small_sb = es.enter_context(
            nc.sbuf_tensor("small_sb", [12, VPC, 128 + W], f32r)
        )
        e_sb = [
            es.enter_context(nc.sbuf_tensor(f"e{v}", [P, NCH, D], E_DT))
            for v in range(VPC)
        ]
        w_sb = [
            es.enter_context(nc.sbuf_tensor(f"w{v}", [P, W], W_DT))
            for v in range(VPC)
        ]
        ones_col = es.enter_context(nc.sbuf_tensor("ones", [P, 1], W_DT))
        warm_l = es.enter_context(nc.sbuf_tensor("warm_l", [P, SC], W_DT))
        warm_r = es.enter_context(nc.sbuf_tensor("warm_r", [P, 512], W_DT))
        scr = es.enter_context(nc.sbuf_tensor("scr", [1, 1], f32))
        rz_sb = [
            es.enter_context(
                nc.sbuf_tensor(f"rz{g}", [64 * len(grp), 1], f32)
            )
            for g, grp in enumerate(groups)
        ]
        o_sb = [
            es.enter_context(
                nc.sbuf_tensor(f"osb{g}", [64 * len(grp), D], W_DT)
            )
            for g, grp in enumerate(groups)
        ]
        arg_ps = [
            es.enter_context(nc.psum_tensor(f"arg{v}", [P, W], f32))
            for v in range(VPC)
        ]
        o_ps = [
            es.enter_context(
                nc.psum_tensor(f"o{g}", [64 * len(grp), D], f32)
            )
            for g, grp in enumerate(groups)
        ]
        warm_ps = es.enter_context(nc.psum_tensor("warm", [SC, 512], f32))
        z_ps = [
            es.enter_context(
                nc.psum_tensor(f"z{g}", [64 * len(grp), 1], f32)
            )
            for g, grp in enumerate(groups)
        ]

        # ---- SYNC: input DMAs up-front (HWDGE ring 1, FIFO); smalls
        #      first (they gate the longest dependent chain).
        nc.sync.dma_start(small_sb[:], small).then_inc(s_small, 16)
        for v in range(VPC):
            nc.sync.dma_start(e_sb[v][:], emb[v]).then_inc(s_emb[v], 16)

        # ---- DVE: constants, then reciprocals per group
        nc.vector.memset(ones_col[:], 1.0).then_inc(s_misc)
        nc.vector.memset(warm_l[:], 0.5).then_inc(s_misc)
        nc.vector.memset(warm_r[:], 0.5).then_inc(s_misc)
        # ---- ACT: dummy exp first => act-table load happens at t=0
        nc.scalar.activation(scr[:], scr[:], Exp, scale=1.0)

        # ---- PE: HAM warm-up matmuls (keep the array busy so the
        #      clock gate opens before the real matmuls), then exponent
        #      matmuls once the smalls have landed.
        nc.tensor.wait_ge(s_misc, 3)
        for _ in range(4):
            nc.tensor.matmul(out=warm_ps[:], lhsT=warm_l[:], rhs=warm_r[:],
                             start=True, stop=True)
        nc.tensor.wait_ge(s_small, 16)
        for v in range(VPC):
            nc.tensor.matmul(
                out=arg_ps[v][:],
                lhsT=small_sb[:, v, 0:128],
                rhs=small_sb[:, v, 128 : 128 + W],
                start=True,
                stop=True,
            ).then_inc(s_arg)

        # ---- ACT: exps (w in bf16, matmul-ready layout)
        for v in range(VPC):
            nc.scalar.wait_ge(s_arg, v + 1)
            nc.scalar.activation(
                w_sb[v][:], arg_ps[v][:], Exp, scale=1.0
            ).then_inc(s_act)

        # ---- PE: weighted sums per group (pairs share the array via
        #      column groups), then tiny z matmuls; inc s_pez per group
        for g, grp in enumerate(groups):
            for gi, v in enumerate(grp):
                nc.tensor.wait_ge(s_act, v + 1)
            last = None
            for j in range(NCH):
                for gi, v in enumerate(grp):
                    last = nc.tensor.matmul(
                        out=z_ps[g][64 * gi : 64 * (gi + 1), :],
                        lhsT=w_sb[v][:, j * SC : (j + 1) * SC],
                        rhs=ones_col[:],
                        start=(j == 0),
                        stop=(j == NCH - 1),
                        skip_group_check=True,
                    )
            last.then_inc(s_pez)
            for gi, v in enumerate(grp):
                nc.tensor.wait_ge(s_emb[v], 16)
            last = None
            for j in range(NCH):
                for gi, v in enumerate(grp):
                    last = nc.tensor.matmul(
                        out=o_ps[g][64 * gi : 64 * (gi + 1), :],
                        lhsT=w_sb[v][:, j * SC : (j + 1) * SC],
                        rhs=e_sb[v][:, j, :],
                        start=(j == 0),
                        stop=(j == NCH - 1),
                        skip_group_check=True,
                    )
            last.then_inc(s_peo)

        # ---- DVE: reciprocals
        for g, grp in enumerate(groups):
            nc.vector.wait_ge(s_pez, g + 1)
            nc.vector.reciprocal(rz_sb[g][:], z_ps[g][:]).then_inc(s_rz)

        # ---- ACT: normalize (Copy with per-partition scale) + out DMA
        #      on HWDGE ring 2 (scalar engine)
        out_incs = 0
        for g, grp in enumerate(groups):
            nc.scalar.wait_ge(s_rz, g + 1)
            nc.scalar.wait_ge(s_peo, g + 1)
            nc.scalar.activation(
                o_sb[g][:], o_ps[g][:], Copy, bias=0.0, scale=rz_sb[g][:]
            )
            v0 = grp[0]
            nc.scalar.dma_start(
                out[v0 : v0 + len(grp)].rearrange("v s d -> (v s) d"),
                o_sb[g][:],
            ).then_inc(s_out, 16)
            out_incs += 16

        # ---- tail: rely on the NEFF-end DGE drain for output
        #      landing; runtime clears the sem range at model start.
        if False:
            nc.sync.wait_ge(s_out, out_incs)
            nc.all_engine_barrier()
            nc.gpsimd.sem_clear(range(sem_lo, sem_hi + 1))
            nc.all_engine_barrier()
        nc.compile()
    return nc


def _get_nc(VPC, N, D):
    key = (VPC, N, D)
    if key not in _NC_CACHE:
        _NC_CACHE[key] = _build_nc(*key)
    return _NC_CACHE[key]


def _shard(chromosome, position, embeddings, centers, log_variances):
    B = chromosome.shape[0]
    K, N, D = embeddings.shape

    counts = np.bincount(chromosome, minlength=K)
    order = np.argsort(chromosome, kind="stable")
    starts = np.zeros(K + 1, dtype=np.int64)
    starts[1:] = np.cumsum(counts)
    sorted_pos = position[order, 0].astype(np.float64)

    vchrs = []
    for k in range(K):
        s, c = starts[k], counts[k]
        while c > 0:
            take = min(c, SC)
            vchrs.append((k, s, take))
            s += take
            c -= take
    nv = len(vchrs)
    VPC = max(1, math.ceil(nv / N_CORES))
    while len(vchrs) < VPC * N_CORES:
        vchrs.append((0, 0, 0))

    t2 = 0.5 * np.exp(-log_variances.astype(np.float64))
    c = centers.astype(np.float64)
    alpha = -t2
    beta = 2.0 * t2 * c
    gamma = -t2 * c * c

    emb_pm = np.ascontiguousarray(
        embeddings.reshape(K, NCH, P, D).transpose(0, 2, 1, 3)
    ).astype(E_NP)

    W = NCH * SC
    in_maps = []
    for i in range(N_CORES):
        emb_i = np.zeros((VPC, P, NCH, D), dtype=E_NP)
        small_i = np.zeros((12, VPC, 128 + W), dtype=np.float32)
        for vloc in range(VPC):
            k, s, cnt = vchrs[i * VPC + vloc]
            if cnt == 0:
                continue
            emb_i[vloc] = emb_pm[k]
            coef = np.zeros((12, 128), dtype=np.float64)
            for j in range(NCH):
                sl = slice(j * P, (j + 1) * P)
                coef[3 * j + 0] = alpha[k, sl]
                coef[3 * j + 1] = beta[k, sl]
                coef[3 * j + 2] = gamma[k, sl]
            small_i[:, vloc, 0:128] = coef
            pb = np.zeros(SC, dtype=np.float64)
            pb[:cnt] = sorted_pos[s : s + cnt]
            basis = np.zeros((12, W), dtype=np.float64)
            for j in range(NCH):
                sl = slice(j * SC, (j + 1) * SC)
                basis[3 * j + 0, sl] = pb * pb
                basis[3 * j + 1, sl] = pb
                basis[3 * j + 2, sl] = 1.0
            small_i[:, vloc, 128:] = basis
        in_maps.append({"emb": emb_i, "small": small_i})
    meta = (B, D, VPC, vchrs, order)
    return in_maps, meta


def kernel(chromosome, position, embeddings, centers, log_variances):
    global LAST_RESULTS
    chromosome = np.asarray(chromosome, dtype=np.int32)
    position = np.asarray(position, dtype=np.float32)
    embeddings = np.asarray(embeddings, dtype=np.float32)
    centers = np.asarray(centers, dtype=np.float32)
    log_variances = np.asarray(log_variances, dtype=np.float32)

    in_maps, meta = _shard(
        chromosome, position, embeddings, centers, log_variances
    )
    B, D, VPC, vchrs, order = meta
    N = embeddings.shape[1]

    nc = _get_nc(VPC, N, D)
    res = run_bass_kernel_spmd(nc, in_maps, core_ids=list(range(N_CORES)))
    LAST_RESULTS = res

    out_full = np.zeros((B, D), dtype=np.float32)
    for i in range(N_CORES):
        o = np.asarray(res.results[i]["out"]).astype(np.float32)
        for vloc in range(VPC):
            k, s, cnt = vchrs[i * VPC + vloc]
            if cnt == 0:
                continue
            idx = order[s : s + cnt]
            out_full[idx] = o[vloc, :cnt]
    return out_full


# revision 5
# speedup vs baseline: 1.6306x; 1.0220x over previous
"""ATACSeq RBF-embedding kernel — raw bacc (manual semaphores), v3.

Same math/host-prep as the tile version (see kernel.py docstring), but
the device program is hand-scheduled with explicit semaphores to avoid
the tile framework's startup/drain overhead:

  SYNC : smalls DMA, 3 embedding DMAs (HWDGE ring 1), final out-wait
  PE   : 3 exponent matmuls -> v0 matmuls -> paired (v1||v2) matmuls
         in separate PE column groups; tiny z matmuls
  ACT  : dummy exp (hoists the ~1.3us table load to t=0), 3 exps,
         normalize-scales (per-partition 1/Z), out DMAs (HWDGE ring 2)
  DVE  : ones memset, reciprocals
  tail : barrier, one sem range-clear, barrier
"""

import math
import sys
import types

import numpy as np
import ml_dtypes

import concourse.bass as bass
from concourse import bacc, mybir
from concourse.bass_utils import run_bass_kernel_spmd
from contextlib import ExitStack


def _ensure_ntff_hook():
    try:
        import antenv.axon_hooks  # noqa: F401

        return
    except Exception:
        pass
    try:
        import antenv

        mod = types.ModuleType("antenv.axon_hooks")
        holder = [None, False]

        def set_axon_ntff_profile_hook(h):
            holder[0] = h
            holder[1] = True

        def get_axon_ntff_profile_hook():
            if not holder[1]:
                holder[1] = True
                try:
                    from trn_agent_boot.trn_boot import (
                        _ntff_profile_via_ctypes,
                    )

                    holder[0] = _ntff_profile_via_ctypes(
                        "/opt/axon/libaxon_pjrt.so"
                    )
                except Exception:
                    holder[0] = None
            return holder[0]

        mod.set_axon_ntff_profile_hook = set_axon_ntff_profile_hook
        mod.get_axon_ntff_profile_hook = get_axon_ntff_profile_hook
        sys.modules["antenv.axon_hooks"] = mod
        antenv.axon_hooks = mod
    except Exception:
        pass


_ensure_ntff_hook()

N_CORES = 8
P = 128
SC = 64
NCH = 4
E_DT = mybir.dt.float8e3      # embedding matrices (DMA halved again)
E_NP = ml_dtypes.float8_e3m4
W_DT = mybir.dt.bfloat16      # RBF weights / ones / outputs

LAST_RESULTS = None
_NC_CACHE = {}


def _build_nc(VPC: int, N: int, D: int):
    f32 = mybir.dt.float32
    f32r = mybir.dt.float32r
    W = NCH * SC
    Exp = mybir.ActivationFunctionType.Exp
    Copy = mybir.ActivationFunctionType.Copy

    nc = bacc.Bacc("TRN2", target_bir_lowering=False, debug=False)
    emb = nc.dram_tensor(
        "emb", [VPC, P, NCH, D], E_DT, kind="ExternalInput"
    ).ap()
    small = nc.dram_tensor(
        "small", [12, VPC, 128 + W], f32r, kind="ExternalInput"
    ).ap()
    out = nc.dram_tensor(
        "out", [VPC, SC, D], W_DT, kind="ExternalOutput"
    ).ap()

    groups = []
    if VPC % 2 == 1:
        groups.append((0,))
        rest = list(range(1, VPC))
    else:
        rest = list(range(VPC))
    for i in range(0, len(rest), 2):
        groups.append(tuple(rest[i : i + 2]))

    s_small = nc.alloc_semaphore("s_small")
    s_emb = [nc.alloc_semaphore(f"s_e{v}") for v in range(VPC)]
    s_misc = nc.alloc_semaphore("s_misc")
    s_arg = nc.alloc_semaphore("s_arg")
    s_peo = nc.alloc_semaphore("s_peo")
    s_act = nc.alloc_semaphore("s_act")
    s_pez = nc.alloc_semaphore("s_pez")
    s_rz = nc.alloc_semaphore("s_rz")
    s_norm = nc.alloc_semaphore("s_norm")
    s_out = nc.alloc_semaphore("s_out")
    sem_lo = s_small.num
    sem_hi = s_out.num

    es = ExitStack()
    with es:
        small_sb = es.enter_context(
            nc.sbuf_tensor("small_sb", [12, VPC, 128 + W], f32r)
        )
        e_sb = [
            es.enter_context(nc.sbuf_tensor(f"e{v}", [P, NCH, D], E_DT))
            for v in range(VPC)
        ]
        w_sb = [
            es.enter_context(nc.sbuf_tensor(f"w{v}", [P, W], W_DT))
            for v in range(VPC)
        ]
        ones_col = es.enter_context(nc.sbuf_tensor("ones", [P, 1], W_DT))
        warm_l = es.enter_context(nc.sbuf_tensor("warm_l", [P, SC], W_DT))
        warm_r = es.enter_context(nc.sbuf_tensor("warm_r", [P, 256], W_DT))
        scr = es.enter_context(nc.sbuf_tensor("scr", [1, 1], f32))
        rz_sb = [
            es.enter_context(
                nc.sbuf_tensor(f"rz{g}", [64 * len(grp), 1], f32)
            )
            for g, grp in enumerate(groups)
        ]
        o_sb = [
            es.enter_context(
                nc.sbuf_tensor(f"osb{g}", [64 * len(grp), D], W_DT)
            )
            for g, grp in enumerate(groups)
        ]
        n_arg = min(VPC, 3)
        arg_pool = [
            es.enter_context(nc.psum_tensor(f"arg{v}", [P, W], f32))
            for v in range(n_arg)
        ]
        arg_ps = [arg_pool[v % n_arg] for v in range(VPC)]
        n_grp = min(len(groups), 2)
        o_pool = [
            es.enter_context(nc.psum_tensor(f"o{g}", [128, D], f32))
            for g in range(n_grp)
        ]
        o_ps = [o_pool[g % n_grp] for g in range(len(groups))]
        z_pool = [
            es.enter_context(nc.psum_tensor(f"z{g}", [128, 1], f32))
            for g in range(n_grp)
        ]
        z_ps = [z_pool[g % n_grp] for g in range(len(groups))]
        warm_ps = arg_pool[0]

        # ---- SYNC: input DMAs up-front (HWDGE ring 1, FIFO); smalls
        #      first (they gate the longest dependent chain).
        nc.sync.dma_start(small_sb[:], small).then_inc(s_small, 16)
        for v in range(VPC):
            nc.sync.dma_start(e_sb[v][:], emb[v]).then_inc(s_emb[v], 16)

        # ---- DVE: constants, then reciprocals per group
        nc.vector.memset(ones_col[:], 1.0).then_inc(s_misc)
        nc.vector.memset(warm_l[:], 0.5).then_inc(s_misc)
        nc.vector.memset(warm_r[:], 0.5).then_inc(s_misc)
        # ---- ACT: dummy exp first => act-table load happens at t=0
        nc.scalar.activation(scr[:], scr[:], Exp, scale=1.0)

        # ---- PE: HAM warm-up matmuls (keep the array busy so the
        #      clock gate opens before the real matmuls), then exponent
        #      matmuls once the smalls have landed.
        nc.tensor.wait_ge(s_misc, 3)
        for _ in range(8):
            nc.tensor.matmul(out=warm_ps[0:SC, :], lhsT=warm_l[:],
                             rhs=warm_r[:], start=True, stop=True)
        nc.tensor.wait_ge(s_small, 16)
        for v in range(VPC):
            if v >= n_arg:
                # bank reuse: exp of the prior occupant must be done
                nc.tensor.wait_ge(s_act, v - n_arg + 1)
            nc.tensor.matmul(
                out=arg_ps[v][:],
                lhsT=small_sb[:, v, 0:128],
                rhs=small_sb[:, v, 128 : 128 + W],
                start=True,
                stop=True,
            ).then_inc(s_arg)

        # ---- ACT: exps (w in bf16, matmul-ready layout)
        for v in range(VPC):
            nc.scalar.wait_ge(s_arg, v + 1)
            nc.scalar.activation(
                w_sb[v][:], arg_ps[v][:], Exp, scale=1.0
            ).then_inc(s_act)

        # ---- PE: weighted sums per group (pairs share the array via
        #      column groups), then tiny z matmuls; inc s_pez per group
        for g, grp in enumerate(groups):
            if g >= n_grp:
                nc.tensor.wait_ge(s_norm, g - n_grp + 1)
                nc.tensor.wait_ge(s_rz, g - n_grp + 1)
            for gi, v in enumerate(grp):
                nc.tensor.wait_ge(s_act, v + 1)
            last = None
            for j in range(NCH):
                for gi, v in enumerate(grp):
                    last = nc.tensor.matmul(
                        out=z_ps[g][64 * gi : 64 * (gi + 1), :],
                        lhsT=w_sb[v][:, j * SC : (j + 1) * SC],
                        rhs=ones_col[:],
                        start=(j == 0),
                        stop=(j == NCH - 1),
                        skip_group_check=True,
                    )
            last.then_inc(s_pez)
            for gi, v in enumerate(grp):
                nc.tensor.wait_ge(s_emb[v], 16)
            last = None
            for j in range(NCH):
                for gi, v in enumerate(grp):
                    last = nc.tensor.matmul(
                        out=o_ps[g][64 * gi : 64 * (gi + 1), :],
                        lhsT=w_sb[v][:, j * SC : (j + 1) * SC],
                        rhs=e_sb[v][:, j, :],
                        start=(j == 0),
                        stop=(j == NCH - 1),
                        skip_group_check=True,
                    )
            last.then_inc(s_peo)

        # ---- DVE: reciprocals
        for g, grp in enumerate(groups):
            nc.vector.wait_ge(s_pez, g + 1)
            nc.vector.reciprocal(
                rz_sb[g][:], z_ps[g][0 : 64 * len(grp), :]
            ).then_inc(s_rz)

        # ---- ACT: normalize (Copy with per-partition scale) + out DMA
        #      on HWDGE ring 2 (scalar engine)
        out_incs = 0
        for g, grp in enumerate(groups):
            nc.scalar.wait_ge(s_rz, g + 1)
            nc.scalar.wait_ge(s_peo, g + 1)
            nc.scalar.activation(
                o_sb[g][:],
                o_ps[g][0 : 64 * len(grp), :],
                Copy,
                bias=0.0,
                scale=rz_sb[g][:],
            ).then_inc(s_norm)
            v0 = grp[0]
            nc.scalar.dma_start(
                out[v0 : v0 + len(grp)].rearrange("v s d -> (v s) d"),
                o_sb[g][:],
            ).then_inc(s_out, 16)
            out_incs += 16

        # ---- tail: rely on the NEFF-end DGE drain for output
        #      landing; runtime clears the sem range at model start.
        if False:
            nc.sync.wait_ge(s_out, out_incs)
            nc.all_engine_barrier()
            nc.gpsimd.sem_clear(range(sem_lo, sem_hi + 1))
            nc.all_engine_barrier()
        nc.compile()
    return nc


def _get_nc(VPC, N, D):
    key = (VPC, N, D)
    if key not in _NC_CACHE:
        _NC_CACHE[key] = _build_nc(*key)
    return _NC_CACHE[key]


def _shard(chromosome, position, embeddings, centers, log_variances):
    B = chromosome.shape[0]
    K, N, D = embeddings.shape

    counts = np.bincount(chromosome, minlength=K)
    order = np.argsort(chromosome, kind="stable")
    starts = np.zeros(K + 1, dtype=np.int64)
    starts[1:] = np.cumsum(counts)
    sorted_pos = position[order, 0].astype(np.float64)

    vchrs = []
    for k in range(K):
        s, c = starts[k], counts[k]
        while c > 0:
            take = min(c, SC)
            vchrs.append((k, s, take))
            s += take
            c -= take
    nv = len(vchrs)
    VPC = max(1, math.ceil(nv / N_CORES))
    while len(vchrs) < VPC * N_CORES:
        vchrs.append((0, 0, 0))

    t2 = 0.5 * np.exp(-log_variances.astype(np.float64))
    c = centers.astype(np.float64)
    alpha = -t2
    beta = 2.0 * t2 * c
    gamma = -t2 * c * c

    emb_pm = np.ascontiguousarray(
        embeddings.reshape(K, NCH, P, D).transpose(0, 2, 1, 3)
    ).astype(E_NP)

    W = NCH * SC
    in_maps = []
    for i in range(N_CORES):
        emb_i = np.zeros((VPC, P, NCH, D), dtype=E_NP)
        small_i = np.zeros((12, VPC, 128 + W), dtype=np.float32)
        for vloc in range(VPC):
            k, s, cnt = vchrs[i * VPC + vloc]
            if cnt == 0:
                continue
            emb_i[vloc] = emb_pm[k]
            coef = np.zeros((12, 128), dtype=np.float64)
            for j in range(NCH):
                sl = slice(j * P, (j + 1) * P)
                coef[3 * j + 0] = alpha[k, sl]
                coef[3 * j + 1] = beta[k, sl]
                coef[3 * j + 2] = gamma[k, sl]
            small_i[:, vloc, 0:128] = coef
            pb = np.zeros(SC, dtype=np.float64)
            pb[:cnt] = sorted_pos[s : s + cnt]
            basis = np.zeros((12, W), dtype=np.float64)
            for j in range(NCH):
                sl = slice(j * SC, (j + 1) * SC)
                basis[3 * j + 0, sl] = pb * pb
                basis[3 * j + 1, sl] = pb
                basis[3 * j + 2, sl] = 1.0
            small_i[:, vloc, 128:] = basis
        in_maps.append({"emb": emb_i, "small": small_i})
    meta = (B, D, VPC, vchrs, order)
    return in_maps, meta


def kernel(chromosome, position, embeddings, centers, log_variances):
    global LAST_RESULTS
    chromosome = np.asarray(chromosome, dtype=np.int32)
    position = np.asarray(position, dtype=np.float32)
    embeddings = np.asarray(embeddings, dtype=np.float32)
    centers = np.asarray(centers, dtype=np.float32)
    log_variances = np.asarray(log_variances, dtype=np.float32)

    in_maps, meta = _shard(
        chromosome, position, embeddings, centers, log_variances
    )
    B, D, VPC, vchrs, order = meta
    N = embeddings.shape[1]

    nc = _get_nc(VPC, N, D)
    res = run_bass_kernel_spmd(nc, in_maps, core_ids=list(range(N_CORES)))
    LAST_RESULTS = res

    out_full = np.zeros((B, D), dtype=np.float32)
    for i in range(N_CORES):
        o = np.asarray(res.results[i]["out"]).astype(np.float32)
        for vloc in range(VPC):
            k, s, cnt = vchrs[i * VPC + vloc]
            if cnt == 0:
                continue
            idx = order[s : s + cnt]
            out_full[idx] = o[vloc, :cnt]
    return out_full
